# revision 9
# baseline (speedup 1.0000x reference)
"""PoH block (3-iter transformer block) on 8 trn2 NeuronCores — v3 (fp8 DoubleRow).

Data-parallel over batch (B=8 -> 1 element/core). All attention matmuls run in
fp8e4m3 with DoubleRow perf mode (k-chunk pairs per instruction). The FFN uses
a 3-term hi/lo split (W_hi*x_hi + W_hi*x_lo + W_lo*x_hi, lo*lo dropped) where
the lo residuals live in e4m3's subnormal range at the same accumulation
scale, giving ~fp16 accuracy at 0.75x the fp16 PE cost. Weights are baked
into the NEFF as Const tensors. Scores contract DH=64 as [32, 2, .] DoubleRow
pairs via an SBUF->SBUF DMA repack of q/k into 32-partition blocks. FFN
matmuls are software-pipelined into the attention exp (ACT) gaps as paced
filler generators; h_hi/h_lo production runs on the GPSIMD (Pool) engine to
keep ACT free for exp.

Scaling: Wq/Wk/Wv x16 (e4m3 normal range) so q/k/v carry x16; exp scale
absorbs 1/256; softmax-denominator ones=16 absorbs v's x16; Wo unscaled
(outcat at true scale); W1/W2 x16 each; FFN drain applies 1/256 via ACT
Copy(scale).

PSUM tag budget (16KB): ps 2x2KB (attn accs/transposes), psf 3x2KB (FFN
chains), sc 1x4KB (scores pair), pvt 1KB, den 1KB.
"""

import base64
import io
from collections import deque

import numpy as np
import ml_dtypes
from contextlib import ExitStack

import concourse.bacc as bacc
import concourse.mybir as mybir
import concourse.tile as tile
from concourse.bass_types import DRamTensorHandle
from concourse.bass_utils import run_bass_kernel_spmd
from concourse.masks import make_identity

F32 = mybir.dt.float32
F32R = mybir.dt.float32r
F16 = mybir.dt.float16
F8 = mybir.dt.float8e4
AF = mybir.ActivationFunctionType
OP = mybir.AluOpType
DRM = mybir.MatmulPerfMode.DoubleRow
E4 = ml_dtypes.float8_e4m3

D = 1024
H = 16
DH = 64
DF = 4096
B = 8
ITERS = 3
EPS = 1e-5
SCALE = 0.125          # 1/sqrt(64)
WS = 16.0              # weight prescale for e4m3 normal range
EXP_SCALE = SCALE / (WS * WS)   # q and k each carry x16

_CACHE = {}
K_STAGE = None  # bisect knob: "proj", "scores", "pv", "noln", None=full


def _inline_const(nc, data, dtype, name):
    """Const DRAM tensor embedded in the NEFF (no per-call traffic)."""
    data = np.ascontiguousarray(data)
    mls = nc._tensor(name, list(data.shape), dtype, kind="Const", type="DRAM")
    buf = io.BytesIO()
    np.save(buf, data, allow_pickle=False)
    mls.file = f"{name}.npy"
    mls.ant_data = base64.standard_b64encode(buf.getvalue()).decode()
    return DRamTensorHandle(name, list(data.shape), dtype)


def build(T, wq8, wk8, wv8, wo8, w1h_np, w1l_np, w2h_np, w2l_np):
    nc = bacc.Bacc("TRN2", target_bir_lowering=False, dynamic_dma_scratch_size=4096)

    NT1 = T // 128     # s chunks of 128
    NTQ = T // 256     # t (query) chunks of 256
    NTH = T // 512     # t chunks of 512 (projection granularity)
    ND = D // 128      # 8
    NF = DF // 128     # 32
    NHEP = H // 2      # 8 head pairs

    z_in = nc.dram_tensor("z_in", [T, D], F32R, kind="ExternalInput")
    wq = _inline_const(nc, wq8, F8, "wq")
    wk = _inline_const(nc, wk8, F8, "wk")
    wv = _inline_const(nc, wv8, F8, "wv")
    wo = _inline_const(nc, wo8, F8, "wo")
    w1h = _inline_const(nc, w1h_np, F8, "w1h")
    w1l = _inline_const(nc, w1l_np, F8, "w1l")
    w2h = _inline_const(nc, w2h_np, F8, "w2h")
    w2l = _inline_const(nc, w2l_np, F8, "w2l")
    z_out = nc.dram_tensor("z_out", [T, D], F16, kind="ExternalOutput")
    z_ln1 = [nc.dram_tensor(f"z_ln1_{i}", [T, D], F16) for i in range(ITERS - 1)]
    z_ln2 = [nc.dram_tensor(f"z_ln2_{i}", [T, D], F16) for i in range(ITERS - 1)]

    with ExitStack() as ctx:
        tc = ctx.enter_context(tile.TileContext(nc))
        ctx.enter_context(nc.allow_low_precision(reason="fp8 pipeline"))
        singles = ctx.enter_context(tc.tile_pool(name="singles", bufs=1))
        work = ctx.enter_context(tc.tile_pool(name="work", bufs=2))
        stats = ctx.enter_context(tc.tile_pool(name="stats", bufs=3))
        ztp = ctx.enter_context(tc.tile_pool(name="ztp", bufs=2))
        zt1p = ctx.enter_context(tc.tile_pool(name="zt1p", bufs=1))
        wres_p = ctx.enter_context(tc.tile_pool(name="wres", bufs=1))
        qks_p = ctx.enter_context(tc.tile_pool(name="qks", bufs=4))
        pack_p = ctx.enter_context(tc.tile_pool(name="packp", bufs=1))
        vg_p = ctx.enter_context(tc.tile_pool(name="vgp", bufs=1))
        outcat_p = ctx.enter_context(tc.tile_pool(name="outcatp", bufs=2))
        et_p = ctx.enter_context(tc.tile_pool(name="etp", bufs=3))
        ocn_p = ctx.enter_context(tc.tile_pool(name="ocnp", bufs=8))
        lnc_p = ctx.enter_context(tc.tile_pool(name="lncp", bufs=2))
        w1c_p = ctx.enter_context(tc.tile_pool(name="w1cp", bufs=4))
        w2c_p = ctx.enter_context(tc.tile_pool(name="w2cp", bufs=4))
        h_p = ctx.enter_context(tc.tile_pool(name="hp", bufs=2))
        ht_p = ctx.enter_context(tc.tile_pool(name="htp", bufs=3))
        fsc_p = ctx.enter_context(tc.tile_pool(name="fscp", bufs=2))
        ps = ctx.enter_context(tc.tile_pool(name="ps", bufs=1, space="PSUM"))

        ident_f = singles.tile([128, 128], F32, name="ident_f")
        make_identity(nc, ident_f)
        ident = singles.tile([128, 128], F16, name="ident")
        nc.vector.tensor_copy(out=ident, in_=ident_f)
        ident_r = singles.tile([128, 128], F32R, name="ident_r")
        nc.vector.tensor_copy(out=ident_r, in_=ident_f)
        eps_t = singles.tile([128, 1], F32, name="eps_t")
        nc.vector.memset(eps_t, EPS)
        ones2 = singles.tile([128, 2, 1], F8, name="ones2")
        nc.vector.memset(ones2, WS)   # den absorbs the v x16

        # resident attention weights (fp8)
        wq_r = wres_p.tile([128, ND, D], F8, name="wq_r")
        wk_r = wres_p.tile([128, ND, D], F8, name="wk_r")
        wv_r = wres_p.tile([128, ND, D], F8, name="wv_r")
        wo_r = wres_p.tile([128, ND, D], F8, name="wo_r")

        # persistent attention tensors
        qt_pack = pack_p.tile([128, 2, 4, T], F8, name="qt_pack")
        kt_pack = pack_p.tile([128, 2, 4, T], F8, name="kt_pack")
        vg = vg_p.tile([128, NT1, H, 64], F8, name="vg")

        def load_resident_weights():
            for wt, wr in ((wq, wq_r), (wk, wk_r), (wv, wv_r), (wo, wo_r)):
                nc.sync.dma_start(out=wr[:, :, :], in_=wt[:, :, :])

        def layernorm_tile(ln_in, z_new):
            """ln_in [128, D] f32 -> z_new (gamma=1, beta=0)."""
            st = stats.tile([128, 2, 6], F32, name="bn", tag="bn")
            for c in range(2):
                nc.vector.bn_stats(out=st[:, c, :], in_=ln_in[:, c * 512:(c + 1) * 512])
            mv = stats.tile([128, 2], F32, name="mv", tag="mv")
            nc.vector.bn_aggr(out=mv, in_=st)
            rstd = stats.tile([128, 1], F32, name="rstd", tag="rstd")
            nc.scalar.activation(out=rstd, in_=mv[:, 1:2], func=AF.Sqrt, bias=eps_t, scale=1.0)
            nc.vector.reciprocal(out=rstd, in_=rstd)
            nc.vector.tensor_scalar(out=z_new, in0=ln_in, scalar1=mv[:, 0:1], scalar2=rstd,
                                    op0=OP.subtract, op1=OP.mult)

        def transpose_z(src_tile, tp, dst_hi, dst_lo, f32r=False, tag="ps"):
            """src_tile [128, D] (t-chunk tp) -> dst_hi[:, dp, tp*128:+128] fp8
            (+ dst_lo = src - hi). 4 transposes share one PSUM slot; generator."""
            idt = ident_r if f32r else ident
            pdt = F32R if f32r else F16
            for g in range(ND // 4):
                pt = ps.tile([128, 4, 128], pdt, name="pt", tag=tag, bufs=PS_BUFS[tag],
                             padded_shape=None if f32r else [128, 4, 256])
                for j in range(4):
                    dp = g * 4 + j
                    nc.tensor.transpose(pt[:, j, :], in_=src_tile[:, dp * 128:(dp + 1) * 128],
                                        identity=idt)
                    yield 55
                sl = (slice(None), slice(g * 4, (g + 1) * 4), slice(tp * 128, (tp + 1) * 128))
                nc.vector.tensor_copy(out=dst_hi[sl], in_=pt)
                if dst_lo is not None:
                    nc.vector.tensor_tensor(out=dst_lo[sl], in0=pt, in1=dst_hi[sl],
                                            op=OP.subtract)
                yield 0

        PS_BUFS = {"ps": 2, "psf": 3}

        # ---- FFN filler machinery ----
        fillers = deque()
        credit = [0.0]

        def pump(budget):
            credit[0] = min(credit[0] + budget, 6000.0)
            while fillers and credit[0] > 0:
                try:
                    credit[0] -= next(fillers[0])
                except StopIteration:
                    fillers.popleft()

        def drain_all():
            while fillers:
                g = fillers.popleft()
                for _ in g:
                    pass
            credit[0] = 0.0

        # ---- initial z0 -> ztA (fp8, d-major) ----
        ztA = ztp.tile([128, ND, T], F8, name="ztA", tag="ztA")
        with tc.tile_pool(name="zip", bufs=4) as zi_p:
            load_resident_weights()
            for tp in range(NT1):
                zi = zi_p.tile([128, D], F32R, name="zi", tag="zi")
                nc.sync.dma_start(out=zi, in_=z_in[tp * 128:(tp + 1) * 128, :])
                for _ in transpose_z(zi, tp, ztA, None, f32r=True):
                    pass

        def ffn_th(it, th, zt1h, zt1l, ztA_next):
            """FFN for t-chunk th (256 rows); yields PE-cost estimates (ns)."""
            h_hi = h_p.tile([128, NF, 256], F8, name="h_hi", tag="h_hi")
            h_lo = h_p.tile([128, NF, 256], F8, name="h_lo", tag="h_lo")
            ts0 = th * 256
            for fblk in range(NF // 4):
                w1ht = w1c_p.tile([128, ND, 512], F8, name="w1ht", tag="w1c")
                w1lt = w1c_p.tile([128, ND, 512], F8, name="w1lt", tag="w1c")
                nc.sync.dma_start(out=w1ht, in_=w1h[:, :, fblk * 512:(fblk + 1) * 512])
                nc.sync.dma_start(out=w1lt, in_=w1l[:, :, fblk * 512:(fblk + 1) * 512])
                accs = []
                ht16 = ht_p.tile([128, 4, 256], F16, name="ht16", tag="ht16")
                for fi2 in range(2):
                    acc = ps.tile([128, 2, 256], F32, name="ah", tag="psf", bufs=3)
                    accs.append(acc)
                    for fj in range(2):
                        fi = fi2 * 2 + fj
                        n = 0
                        for wt, xt in ((w1ht, zt1h), (w1ht, zt1l), (w1lt, zt1h)):
                            for dpp in range(ND // 2):
                                nc.tensor.matmul(
                                    acc[:, fj, :],
                                    lhsT=wt[:, 2 * dpp:2 * dpp + 2, fi * 128:(fi + 1) * 128],
                                    rhs=xt[:, 2 * dpp:2 * dpp + 2, ts0:ts0 + 256],
                                    start=(n == 0), stop=(n == 11), perf_mode=DRM)
                                n += 1
                                yield 55
                    nc.scalar.activation(out=ht16[:, 2 * fi2:2 * fi2 + 2, :],
                                         in_=acc, func=AF.Relu, scale=1.0)
                    yield 0
                fc0 = fblk * 4
                nc.gpsimd.tensor_copy(out=h_hi[:, fc0:fc0 + 4, :], in_=ht16)
                nc.gpsimd.tensor_tensor(out=h_lo[:, fc0:fc0 + 4, :], in0=ht16,
                                        in1=h_hi[:, fc0:fc0 + 4, :], op=OP.subtract)
                yield 0
            # W2: per dq, two 128-row chains (ti) interleaved across fc
            fscs = []
            for dq in range(2):
                accs2 = [ps.tile([128, 512], F32, name="af", tag="psf", bufs=3)
                         for _ in range(2)]
                n = [0, 0]
                for fg in range(NF // 4):
                    w2ht = w2c_p.tile([128, 4, 512], F8, name="w2ht", tag="w2c")
                    w2lt = w2c_p.tile([128, 4, 512], F8, name="w2lt", tag="w2c")
                    nc.sync.dma_start(out=w2ht, in_=w2h[:, fg * 4:(fg + 1) * 4,
                                                       dq * 512:(dq + 1) * 512])
                    nc.sync.dma_start(out=w2lt, in_=w2l[:, fg * 4:(fg + 1) * 4,
                                                       dq * 512:(dq + 1) * 512])
                    for fcp in range(2):
                        fc2 = fg * 4 + fcp * 2
                        for ti in range(2):
                            for wt, xt in ((w2ht, h_hi), (w2ht, h_lo), (w2lt, h_hi)):
                                nc.tensor.matmul(
                                    accs2[ti],
                                    lhsT=xt[:, fc2:fc2 + 2, ti * 128:(ti + 1) * 128],
                                    rhs=wt[:, fcp * 2:fcp * 2 + 2, :],
                                    start=(n[ti] == 0), stop=(n[ti] == 47),
                                    perf_mode=DRM)
                                n[ti] += 1
                                yield 110
                fsc = fsc_p.tile([128, 2, 512], F16, name="fsc", tag="fsc")
                for ti in range(2):
                    nc.scalar.activation(out=fsc[:, ti, :], in_=accs2[ti], func=AF.Copy,
                                         scale=1.0 / (WS * WS))
                fscs.append(fsc)
                yield 0
            # residual + LN2 + transposes -> ztA_next
            for ti in range(2):
                tp = th * 2 + ti
                zp = work.tile([128, D], F16, name="zp2", tag="zres16", bufs=3)
                nc.sync.dma_start(out=zp, in_=z_ln1[it][tp * 128:(tp + 1) * 128, :])
                ln_in = lnc_p.tile([128, D], F32, name="ln_in2", tag="lnc")
                for dq in range(2):
                    nc.vector.tensor_add(out=ln_in[:, dq * 512:(dq + 1) * 512],
                                         in0=zp[:, dq * 512:(dq + 1) * 512],
                                         in1=fscs[dq][:, ti, :])
                z_new = work.tile([128, D], F16, name="z_new2", tag="z_new", bufs=3)
                layernorm_tile(ln_in, z_new)
                nc.sync.dma_start(out=z_ln2[it][tp * 128:(tp + 1) * 128, :], in_=z_new)
                yield 40
                for c in transpose_z(z_new, tp, ztA_next, None, tag="psf"):
                    yield c

        for it in range(ITERS):
            last = it == ITERS - 1
            if not last:
                ztA_next = ztp.tile([128, ND, T], F8, name="ztAn", tag="ztA")
                zt1h = zt1p.tile([128, ND, T], F8, name="zt1h", tag="zt1h")
                zt1l = zt1p.tile([128, ND, T], F8, name="zt1l", tag="zt1l")
            else:
                ztA_next = zt1h = zt1l = None

            # ---- projections (DR over dp pairs) ----
            def qk_block(wr, dst_pack, hep, tqh):
                acc = ps.tile([128, 512], F32, name="acq", tag="ps", bufs=2)
                for dpp in range(ND // 2):
                    nc.tensor.matmul(acc, lhsT=wr[:, 2 * dpp:2 * dpp + 2,
                                                  hep * 128:(hep + 1) * 128],
                                     rhs=ztA[:, 2 * dpp:2 * dpp + 2,
                                             tqh * 512:(tqh + 1) * 512],
                                     start=(dpp == 0), stop=(dpp == ND // 2 - 1),
                                     perf_mode=DRM)
                    yield 110
                stg = qks_p.tile([128, 512], F8, name="stg", tag="stg")
                nc.vector.tensor_copy(out=stg, in_=acc)
                # pack: head h = 2*hep+hh -> partitions [32*(2*(hep%2)+hh), +32),
                # hslot = hep//2, pair dim = 32-sub
                for hh in range(2):
                    Bp = 32 * (2 * (hep % 2) + hh)
                    for sub in range(2):
                        nc.sync.dma_start(
                            out=dst_pack[Bp:Bp + 32, sub, hep // 2,
                                         tqh * 512:(tqh + 1) * 512],
                            in_=stg[64 * hh + 32 * sub:64 * hh + 32 * sub + 32, :])
                yield 40

            def v_block(sp, half):
                acc = ps.tile([128, 512], F32, name="acv", tag="ps", bufs=2)
                for dpp in range(ND // 2):
                    nc.tensor.matmul(
                        acc, lhsT=ztA[:, 2 * dpp:2 * dpp + 2, sp * 128:(sp + 1) * 128],
                        rhs=wv_r[:, 2 * dpp:2 * dpp + 2, half * 512:(half + 1) * 512],
                        start=(dpp == 0), stop=(dpp == ND // 2 - 1), perf_mode=DRM)
                    yield 110
                nc.vector.tensor_copy(
                    out=vg[:, sp, half * 8:(half + 1) * 8, :],
                    in_=acc.rearrange("p (h e) -> p h e", e=64))
                yield 40

            for hep in range(NHEP):
                for tqh in range(NTH):
                    for c in qk_block(wq_r, qt_pack, hep, tqh):
                        pump(c)
                    for c in qk_block(wk_r, kt_pack, hep, tqh):
                        pump(c)
            for sp in range(NT1):
                for half in range(2):
                    for c in v_block(sp, half):
                        pump(c)

            # ---- attention core ----
            z_prev = z_in if it == 0 else z_ln2[it - 1]
            dst_ln1 = z_out if last else z_ln1[it]
            state = {}

            def attn_scores_exp(tq, hep):
                et = et_p.tile([128, NT1, 2, 256], F8, name="et", tag="et")
                state["et"] = et
                for spp in range(NT1 // 2):
                    # hh-major: each PSUM bank sees a single tile_position
                    sc = ps.tile([128, 2, 2, 256], F32, name="sc", tag="sc", bufs=1)
                    for hh in range(2):
                        for spi in range(2):
                            sp = 2 * spp + spi
                            Bp = 32 * (2 * (hep % 2) + hh)
                            hslot = hep // 2
                            nc.tensor.matmul(
                                sc[:, hh, spi, :],
                                lhsT=kt_pack[Bp:Bp + 32, :, hslot,
                                             sp * 128:(sp + 1) * 128],
                                rhs=qt_pack[Bp:Bp + 32, :, hslot,
                                            tq * 256:(tq + 1) * 256],
                                start=True, stop=True, perf_mode=DRM,
                                tile_position=(Bp, 0))
                            yield 55
                    if K_STAGE == "scmm":
                        dr8 = et_p.tile([128, 2, 2, 256], F8, name="dr8", tag="dr8",
                                        bufs=2)
                        nc.vector.tensor_copy(out=dr8, in_=sc)
                    else:
                        nc.scalar.activation(
                            out=et[:, 2 * spp:2 * spp + 2, :, :].rearrange(
                                "p s h t -> p h s t"),
                            in_=sc, func=AF.Exp, scale=EXP_SCALE)
                    yield 900

            def attn_pv(tq, hep, et, outcat):
                pvd = ps.tile([128, 2, 2, 2, 64], F32, name="pvd", tag="pvd", bufs=1)
                pvt = pvd[:, 0]
                den = pvd[:, 1]
                for tqc in range(2):
                    for hh in range(2):
                        for spp in range(NT1 // 2):
                            nc.tensor.matmul(
                                pvt[:, tqc, hh, :],
                                lhsT=et[:, 2 * spp:2 * spp + 2, hh,
                                        tqc * 128:(tqc + 1) * 128],
                                rhs=vg[:, 2 * spp:2 * spp + 2, 2 * hep + hh, :],
                                start=(spp == 0), stop=(spp == NT1 // 2 - 1),
                                perf_mode=DRM)
                            yield 28
                        for spp in range(NT1 // 2):
                            nc.tensor.matmul(
                                den[:, tqc, hh, 0:1],
                                lhsT=et[:, 2 * spp:2 * spp + 2, hh,
                                        tqc * 128:(tqc + 1) * 128],
                                rhs=ones2,
                                start=(spp == 0), stop=(spp == NT1 // 2 - 1),
                                perf_mode=DRM)
                            yield 4
                rec = stats.tile([128, 2, 2], F32, name="rec", tag="rec")
                nc.vector.reciprocal(out=rec, in_=den[:, :, :, 0])
                ocns = {}
                for hh in range(2):
                    for tqc in range(2):
                        ocn = ocn_p.tile([128, 64], F16, name="ocn", tag="ocn")
                        nc.vector.tensor_scalar_mul(
                            out=ocn, in0=pvt[:, tqc, hh, :],
                            scalar1=rec[:, tqc, hh:hh + 1])
                        ocns[(hh, tqc)] = ocn
                for hh in range(2):
                    tr = ps.tile([64, 2, 128], F16, name="tr", tag="ps", bufs=2,
                                 padded_shape=[64, 2, 256])
                    for tqc in range(2):
                        nc.tensor.transpose(tr[:, tqc, :], in_=ocns[(hh, tqc)],
                                            identity=ident)
                        yield 55
                    nc.vector.tensor_copy(
                        out=outcat[hh * 64:(hh + 1) * 64, hep, :],
                        in_=tr)
                    yield 0

            def wo_ln(tq, outcat):
                for tpq in range(2):
                    accs = []
                    for dq in range(2):
                        accs.append(ps.tile([128, 512], F32, name="awo", tag="ps",
                                            bufs=2))
                    for dq in range(2):
                        for hpp in range(NHEP // 2):
                            nc.tensor.matmul(
                                accs[dq],
                                lhsT=outcat[:, 2 * hpp:2 * hpp + 2,
                                            tpq * 128:(tpq + 1) * 128],
                                rhs=wo_r[:, 2 * hpp:2 * hpp + 2,
                                         dq * 512:(dq + 1) * 512],
                                start=(hpp == 0), stop=(hpp == NHEP // 2 - 1),
                                perf_mode=DRM)
                            yield 110
                    tp = tq * 2 + tpq
                    if it == 0:
                        zp = work.tile([128, D], F32R, name="zp", tag="zres", bufs=2)
                    else:
                        zp = work.tile([128, D], F16, name="zp16", tag="zres16", bufs=3)
                    nc.sync.dma_start(out=zp, in_=z_prev[tp * 128:(tp + 1) * 128, :])
                    ln_in = lnc_p.tile([128, D], F32, name="ln_in", tag="lnc")
                    for dq in range(2):
                        nc.vector.tensor_add(out=ln_in[:, dq * 512:(dq + 1) * 512],
                                             in0=zp[:, dq * 512:(dq + 1) * 512],
                                             in1=accs[dq])
                    z_new = work.tile([128, D], F16, name="z_new", tag="z_new", bufs=3)
                    layernorm_tile(ln_in, z_new)
                    nc.sync.dma_start(out=dst_ln1[tp * 128:(tp + 1) * 128, :], in_=z_new)
                    yield 40
                    if not last:
                        for c in transpose_z(z_new, tp, zt1h, zt1l):
                            yield c

            if K_STAGE == "proj":
                dummy = work.tile([128, D], F16, name="dummy", tag="z_new", bufs=3)
                nc.vector.memset(dummy, 0.0)
                for tp in range(NT1):
                    nc.sync.dma_start(out=z_out[tp * 128:(tp + 1) * 128, :], in_=dummy)
                break
            for tq in range(NTQ):
                outcat = outcat_p.tile([128, NHEP, 256], F8, name="outcat", tag="outcat")
                prev_pv = None
                for hep in range(NHEP):
                    for c in attn_scores_exp(tq, hep):
                        pump(c)
                    et = state["et"]
                    if K_STAGE in ("scores", "scmm"):
                        continue
                    if prev_pv is not None:
                        for c in prev_pv:
                            pump(c)
                    prev_pv = attn_pv(tq, hep, et, outcat)
                    pump(1200)
                if prev_pv is not None:
                    for c in prev_pv:
                        pump(c)
                if K_STAGE in ("scores", "scmm", "pv"):
                    continue
                for c in wo_ln(tq, outcat):
                    pump(c)
                if not last:
                    fillers.append(ffn_th(it, tq, zt1h, zt1l, ztA_next))
            if K_STAGE in ("scores", "scmm", "pv"):
                dummy = work.tile([128, D], F16, name="dummy", tag="z_new", bufs=3)
                nc.vector.memset(dummy, 0.0)
                for tp in range(NT1):
                    nc.sync.dma_start(out=z_out[tp * 128:(tp + 1) * 128, :], in_=dummy)
                break

            if last:
                break
            drain_all()
            ztA = ztA_next

    nc.compile()
    return nc


def _prep_weights(Wq, Wk, Wv, Wo, W1, W2):
    def pm_e4(a):
        R, C = a.shape
        return np.ascontiguousarray(
            np.ascontiguousarray(a).reshape(R // 128, 128, C).transpose(1, 0, 2))

    def flat(w):
        return np.ascontiguousarray(
            w.transpose(1, 0, 2).reshape(D, D)).astype(np.float32)

    wq8 = pm_e4((flat(Wq) * WS).astype(E4))
    wk8 = pm_e4((flat(Wk) * WS).astype(E4))
    wv8 = pm_e4((flat(Wv) * WS).astype(E4))
    wo8 = pm_e4(Wo.astype(np.float32).astype(E4))      # unscaled
    W1s = W1.astype(np.float32) * WS
    W1hi = W1s.astype(E4)
    W1lo = (W1s - W1hi.astype(np.float32)).astype(E4)
    W2s = W2.astype(np.float32) * WS
    W2hi = W2s.astype(E4)
    W2lo = (W2s - W2hi.astype(np.float32)).astype(E4)
    return (wq8, wk8, wv8, wo8, pm_e4(W1hi), pm_e4(W1lo), pm_e4(W2hi), pm_e4(W2lo))


def kernel(**inputs):
    z = np.asarray(inputs["z"], dtype=np.float32)
    for nm in ("bq", "bk", "bv", "bo", "b1", "b2", "be1", "be2"):
        assert not np.any(np.asarray(inputs[nm])), f"{nm} must be zero (specialized kernel)"
    for nm in ("g1", "g2"):
        assert np.all(np.asarray(inputs[nm]) == 1.0), f"{nm} must be ones (specialized kernel)"

    T = z.shape[1]
    raw_ws = tuple(np.asarray(inputs[nm]) for nm in ("Wq", "Wk", "Wv", "Wo", "W1", "W2"))
    ent = _CACHE.get(T)
    if ent is None or not all(np.array_equal(a, b) for a, b in zip(ent[1], raw_ws)):
        prepped = _prep_weights(*raw_ws)
        ent = (build(T, *prepped), tuple(np.copy(w) for w in raw_ws))
        _CACHE[T] = ent
    nc = ent[0]

    in_maps = [{"z_in": np.ascontiguousarray(z[c])} for c in range(B)]
    res = run_bass_kernel_spmd(nc, in_maps, core_ids=list(range(B)))
    return np.stack([res.results[c]["z_out"] for c in range(B)]).astype(np.float32)


# revision 30
# speedup vs baseline: 1.3236x; 1.3236x over previous
"""PoH block (3-iter transformer block) on 8 trn2 NeuronCores — v3 (fp8 DoubleRow).

Data-parallel over batch (B=8 -> 1 element/core). All attention matmuls run in
fp8e4m3 with DoubleRow perf mode (k-chunk pairs per instruction). The FFN uses
a 3-term hi/lo split (W_hi*x_hi + W_hi*x_lo + W_lo*x_hi, lo*lo dropped) where
the lo residuals live in e4m3's subnormal range at the same accumulation
scale, giving ~fp16 accuracy at 0.75x the fp16 PE cost. Weights are baked
into the NEFF as Const tensors. Scores contract DH=64 as [32, 2, .] DoubleRow
pairs via an SBUF->SBUF DMA repack of q/k into 32-partition blocks. FFN
matmuls are software-pipelined into the attention exp (ACT) gaps as paced
filler generators; h_hi/h_lo production runs on the GPSIMD (Pool) engine to
keep ACT free for exp.

Scaling: Wq/Wk/Wv x16 (e4m3 normal range) so q/k/v carry x16; exp scale
absorbs 1/256; softmax-denominator ones=16 absorbs v's x16; Wo unscaled
(outcat at true scale); W1/W2 x16 each; FFN drain applies 1/256 via ACT
Copy(scale).

PSUM tag budget (16KB): ps 2x2KB (attn accs/transposes), psf 3x2KB (FFN
chains), sc 1x4KB (scores pair), pvt 1KB, den 1KB.
"""

import base64
import io
from collections import deque

import numpy as np
import ml_dtypes
from contextlib import ExitStack

import concourse.bacc as bacc
import concourse.mybir as mybir
import concourse.tile as tile
from concourse.bass_types import DRamTensorHandle
from concourse.bass_utils import run_bass_kernel_spmd
from concourse.masks import make_identity

F32 = mybir.dt.float32
F32R = mybir.dt.float32r
F16 = mybir.dt.float16
F8 = mybir.dt.float8e4
AF = mybir.ActivationFunctionType
OP = mybir.AluOpType
DRM = mybir.MatmulPerfMode.DoubleRow
E4 = ml_dtypes.float8_e4m3

D = 1024
H = 16
DH = 64
DF = 4096
B = 8
ITERS = 3
EPS = 1e-5
SCALE = 0.125          # 1/sqrt(64)
WS = 16.0              # weight prescale for e4m3 normal range
EXP_SCALE = SCALE / (WS * WS)   # q and k each carry x16

_CACHE = {}
K_STAGE = None  # bisect knob: "proj", "scores", "pv", "noln", None=full


def _inline_const(nc, data, dtype, name):
    """Const DRAM tensor embedded in the NEFF (no per-call traffic)."""
    data = np.ascontiguousarray(data)
    mls = nc._tensor(name, list(data.shape), dtype, kind="Const", type="DRAM")
    buf = io.BytesIO()
    np.save(buf, data, allow_pickle=False)
    mls.file = f"{name}.npy"
    mls.ant_data = base64.standard_b64encode(buf.getvalue()).decode()
    return DRamTensorHandle(name, list(data.shape), dtype)


def build(T, wq8, wk8, wv8, wo8, w1h_np, w1l_np, w2h_np, w2l_np):
    nc = bacc.Bacc("TRN2", target_bir_lowering=False, dynamic_dma_scratch_size=4096)

    NT1 = T // 128     # s chunks of 128
    NTQ = T // 256     # t (query) chunks of 256
    NTH = T // 512     # t chunks of 512 (projection granularity)
    ND = D // 128      # 8
    NF = DF // 128     # 32
    NHEP = H // 2      # 8 head pairs

    z_in = nc.dram_tensor("z_in", [T, D], F32R, kind="ExternalInput")
    wq = _inline_const(nc, wq8, F8, "wq")
    wk = _inline_const(nc, wk8, F8, "wk")
    wv = _inline_const(nc, wv8, F8, "wv")
    wo = _inline_const(nc, wo8, F8, "wo")
    w1h = _inline_const(nc, w1h_np, F8, "w1h")
    w1l = _inline_const(nc, w1l_np, F8, "w1l")
    w2h = _inline_const(nc, w2h_np, F8, "w2h")
    w2l = _inline_const(nc, w2l_np, F8, "w2l")
    z_out = nc.dram_tensor("z_out", [T, D], F16, kind="ExternalOutput")
    z_ln1 = [nc.dram_tensor(f"z_ln1_{i}", [T, D], F16) for i in range(ITERS - 1)]
    z_ln2 = [nc.dram_tensor(f"z_ln2_{i}", [T, D], F16) for i in range(ITERS - 1)]

    with ExitStack() as ctx:
        tc = ctx.enter_context(tile.TileContext(nc))
        ctx.enter_context(nc.allow_low_precision(reason="fp8 pipeline"))
        singles = ctx.enter_context(tc.tile_pool(name="singles", bufs=1))
        work = ctx.enter_context(tc.tile_pool(name="work", bufs=2))
        stats = ctx.enter_context(tc.tile_pool(name="stats", bufs=3))
        ztp = ctx.enter_context(tc.tile_pool(name="ztp", bufs=2))
        zt1p = ctx.enter_context(tc.tile_pool(name="zt1p", bufs=1))
        wres_p = ctx.enter_context(tc.tile_pool(name="wres", bufs=1))
        qks_p = ctx.enter_context(tc.tile_pool(name="qks", bufs=2))
        pack_p = ctx.enter_context(tc.tile_pool(name="packp", bufs=1))
        vg_p = ctx.enter_context(tc.tile_pool(name="vgp", bufs=1))
        outcat_p = ctx.enter_context(tc.tile_pool(name="outcatp", bufs=2))
        et_p = ctx.enter_context(tc.tile_pool(name="etp", bufs=3))
        ocn_p = ctx.enter_context(tc.tile_pool(name="ocnp", bufs=6))
        lnc_p = ctx.enter_context(tc.tile_pool(name="lncp", bufs=2))
        w1c_p = ctx.enter_context(tc.tile_pool(name="w1cp", bufs=4))
        w2c_p = ctx.enter_context(tc.tile_pool(name="w2cp", bufs=4))
        h_p = ctx.enter_context(tc.tile_pool(name="hp", bufs=1))
        ht_p = ctx.enter_context(tc.tile_pool(name="htp", bufs=2))
        fsc_p = ctx.enter_context(tc.tile_pool(name="fscp", bufs=2))
        ps = ctx.enter_context(tc.tile_pool(name="ps", bufs=1, space="PSUM"))

        ident_f = singles.tile([128, 128], F32, name="ident_f")
        make_identity(nc, ident_f)
        ident = singles.tile([128, 128], F16, name="ident")
        nc.vector.tensor_copy(out=ident, in_=ident_f)
        ident_r = singles.tile([128, 128], F32R, name="ident_r")
        nc.vector.tensor_copy(out=ident_r, in_=ident_f)
        eps_t = singles.tile([128, 1], F32, name="eps_t")
        nc.vector.memset(eps_t, EPS)
        ones2 = singles.tile([128, 2, 1], F8, name="ones2")
        nc.vector.memset(ones2, WS)   # den absorbs the v x16

        # resident attention weights (fp8)
        wq_r = wres_p.tile([128, ND, D], F8, name="wq_r")
        wk_r = wres_p.tile([128, ND, D], F8, name="wk_r")
        wv_r = wres_p.tile([128, ND, D], F8, name="wv_r")
        wo_r = wres_p.tile([128, ND, D], F8, name="wo_r")

        # persistent attention tensors
        qt_pack = pack_p.tile([128, 2, 4, T], F8, name="qt_pack")
        kt_pack = pack_p.tile([128, 2, 4, T], F8, name="kt_pack")
        vg = vg_p.tile([128, NT1, H, 64], F8, name="vg")

        def load_resident_weights():
            for wt, wr in ((wq, wq_r), (wk, wk_r), (wv, wv_r), (wo, wo_r)):
                nc.sync.dma_start(out=wr[:, :, :], in_=wt[:, :, :])

        def layernorm_tile(ln_in, z_new, on_dve=False):
            """ln_in [128, D] f32 -> z_new (gamma=1, beta=0)."""
            st = stats.tile([128, 2, 6], F32, name="bn", tag="bn")
            for c in range(2):
                nc.vector.bn_stats(out=st[:, c, :], in_=ln_in[:, c * 512:(c + 1) * 512])
            mv = stats.tile([128, 2], F32, name="mv", tag="mv")
            nc.vector.bn_aggr(out=mv, in_=st)
            rstd = stats.tile([128, 1], F32, name="rstd", tag="rstd")
            nc.scalar.activation(out=rstd, in_=mv[:, 1:2], func=AF.Sqrt, bias=eps_t, scale=1.0)
            nc.vector.reciprocal(out=rstd, in_=rstd)
            eng = nc.vector if on_dve else nc.gpsimd
            eng.tensor_scalar(out=z_new, in0=ln_in, scalar1=mv[:, 0:1],
                              scalar2=rstd, op0=OP.subtract, op1=OP.mult)

        def transpose_z(src_tile, tp, dst_hi, dst_lo, f32r=False, tag="ps"):
            """src_tile [128, D] (t-chunk tp) -> dst_hi[:, dp, tp*128:+128] fp8
            (+ dst_lo = src - hi). 4 transposes share one PSUM slot; generator."""
            idt = ident_r if f32r else ident
            pdt = F32R if f32r else F16
            for g in range(ND // 4):
                pt = ps.tile([128, 4, 128], pdt, name="pt", tag=tag, bufs=PS_BUFS[tag],
                             padded_shape=None if f32r else [128, 4, 256])
                for j in range(4):
                    dp = g * 4 + j
                    nc.tensor.transpose(pt[:, j, :], in_=src_tile[:, dp * 128:(dp + 1) * 128],
                                        identity=idt)
                    yield 55
                sl = (slice(None), slice(g * 4, (g + 1) * 4), slice(tp * 128, (tp + 1) * 128))
                nc.vector.tensor_copy(out=dst_hi[sl], in_=pt)
                if dst_lo is not None:
                    nc.vector.tensor_tensor(out=dst_lo[sl], in0=pt, in1=dst_hi[sl],
                                            op=OP.subtract)
                yield 0

        PS_BUFS = {"ps": 2, "psf": 2}

        # ---- FFN filler machinery ----
        fillers = deque()
        credit = [0.0]

        def pump(budget):
            credit[0] = min(credit[0] + budget, 6000.0)
            while fillers and credit[0] > 0:
                try:
                    credit[0] -= next(fillers[0])
                except StopIteration:
                    fillers.popleft()

        def drain_all():
            while fillers:
                g = fillers.popleft()
                for _ in g:
                    pass
            credit[0] = 0.0

        # ---- initial z0 -> ztA (fp8, d-major) ----
        ztA = ztp.tile([128, ND, T], F8, name="ztA", tag="ztA")
        load_resident_weights()
        for tp in range(NT1):
            zi = lnc_p.tile([128, D], F32R, name="zi", tag="lnc")
            nc.sync.dma_start(out=zi, in_=z_in[tp * 128:(tp + 1) * 128, :])
            for _ in transpose_z(zi, tp, ztA, None, f32r=True):
                pass

        def ffn_th(it, th, zt1h, zt1l, ztA_next, late=False):
            """FFN for t-chunk th (256 rows); yields PE-cost estimates (ns).
            late: runs in the post-attention valley -> use ACT (idle there) for
            h production and DVE for LN2 instead of the loaded Pool."""
            h_hi = h_p.tile([128, NF, 256], F8, name="h_hi", tag="h_hi")
            h_lo = h_p.tile([128, NF, 256], F8, name="h_lo", tag="h_lo")
            ts0 = th * 256
            w1ht2 = w1lt2 = None
            for fblk in range(NF // 4):
                if fblk % 2 == 0:
                    w1ht2 = w1c_p.tile([128, ND, 1024], F8, name="w1ht", tag="w1c")
                    w1lt2 = w1c_p.tile([128, ND, 1024], F8, name="w1lt", tag="w1c")
                    nc.sync.dma_start(out=w1ht2,
                                      in_=w1h[:, :, fblk * 512:(fblk + 2) * 512])
                    nc.sync.dma_start(out=w1lt2,
                                      in_=w1l[:, :, fblk * 512:(fblk + 2) * 512])
                off = (fblk % 2) * 512
                w1ht = w1ht2[:, :, off:off + 512]
                w1lt = w1lt2[:, :, off:off + 512]
                fc0 = fblk * 4
                ht16 = None if late else ht_p.tile([128, 4, 256], F16,
                                                   name="ht16", tag="ht16")
                for fi2 in range(2):
                    acc = ps.tile([128, 2, 256], F32, name="ah", tag="psf", bufs=2)
                    for fj in range(2):
                        fi = fi2 * 2 + fj
                        n = 0
                        for wt, xt in ((w1ht, zt1h), (w1ht, zt1l), (w1lt, zt1h)):
                            for dpp in range(ND // 2):
                                nc.tensor.matmul(
                                    acc[:, fj, :],
                                    lhsT=wt[:, 2 * dpp:2 * dpp + 2, fi * 128:(fi + 1) * 128],
                                    rhs=xt[:, 2 * dpp:2 * dpp + 2, ts0:ts0 + 256],
                                    start=(n == 0), stop=(n == 11), perf_mode=DRM)
                                n += 1
                                yield 55
                    fr = slice(fc0 + 2 * fi2, fc0 + 2 * fi2 + 2)
                    if late:
                        nc.scalar.activation(out=h_hi[:, fr, :], in_=acc,
                                             func=AF.Relu, scale=1.0)
                        nc.vector.scalar_tensor_tensor(
                            out=h_lo[:, fr, :], in0=acc, scalar=0.0,
                            in1=h_hi[:, fr, :], op0=OP.max, op1=OP.subtract)
                    else:
                        nc.scalar.activation(out=ht16[:, 2 * fi2:2 * fi2 + 2, :],
                                             in_=acc, func=AF.Relu, scale=1.0)
                    yield 0
                if not late:
                    nc.gpsimd.tensor_copy(out=h_hi[:, fc0:fc0 + 4, :], in_=ht16)
                    nc.gpsimd.tensor_tensor(out=h_lo[:, fc0:fc0 + 4, :], in0=ht16,
                                            in1=h_hi[:, fc0:fc0 + 4, :],
                                            op=OP.subtract)
                    yield 0
            # W2: per dq, two 128-row chains (ti) interleaved across fc
            zp2t2 = work.tile([128, 2, D], F16, name="zp2", tag="zres16", bufs=2)
            nc.sync.dma_start(
                out=zp2t2,
                in_=z_ln1[it][th * 256:(th + 1) * 256, :].rearrange(
                    "(two p) d -> p two d", two=2))
            zps2 = [zp2t2[:, 0, :], zp2t2[:, 1, :]]
            fscs = []
            for dq in range(2):
                accs2 = [ps.tile([128, 512], F32, name="af", tag="psf", bufs=2)
                         for _ in range(2)]
                n = [0, 0]
                w2ht2 = w2lt2 = None
                for fg in range(NF // 4):
                    if fg % 2 == 0:
                        w2ht2 = w2c_p.tile([128, 8, 512], F8, name="w2ht", tag="w2c")
                        w2lt2 = w2c_p.tile([128, 8, 512], F8, name="w2lt", tag="w2c")
                        nc.sync.dma_start(out=w2ht2,
                                          in_=w2h[:, fg * 4:(fg + 2) * 4,
                                                  dq * 512:(dq + 1) * 512])
                        nc.sync.dma_start(out=w2lt2,
                                          in_=w2l[:, fg * 4:(fg + 2) * 4,
                                                  dq * 512:(dq + 1) * 512])
                    offg = (fg % 2) * 4
                    w2ht = w2ht2[:, offg:offg + 4, :]
                    w2lt = w2lt2[:, offg:offg + 4, :]
                    for fcp in range(2):
                        fc2 = fg * 4 + fcp * 2
                        for ti in range(2):
                            for wt, xt in ((w2ht, h_hi), (w2ht, h_lo), (w2lt, h_hi)):
                                nc.tensor.matmul(
                                    accs2[ti],
                                    lhsT=xt[:, fc2:fc2 + 2, ti * 128:(ti + 1) * 128],
                                    rhs=wt[:, fcp * 2:fcp * 2 + 2, :],
                                    start=(n[ti] == 0), stop=(n[ti] == 47),
                                    perf_mode=DRM)
                                n[ti] += 1
                                yield 110
                fsc = fsc_p.tile([128, 2, 512], F16, name="fsc", tag="fsc")
                for ti in range(2):
                    nc.vector.tensor_scalar(out=fsc[:, ti, :], in0=accs2[ti],
                                            scalar1=1.0 / (WS * WS), scalar2=None,
                                            op0=OP.mult)
                fscs.append(fsc)
                yield 0
            # residual + LN2 + transposes -> ztA_next
            for ti in range(2):
                tp = th * 2 + ti
                zp = zps2[ti]
                ln_in = lnc_p.tile([128, D], F32, name="ln_in2", tag="lnc")
                for dq in range(2):
                    nc.vector.tensor_add(out=ln_in[:, dq * 512:(dq + 1) * 512],
                                         in0=zp[:, dq * 512:(dq + 1) * 512],
                                         in1=fscs[dq][:, ti, :])
                z_new = work.tile([128, D], F16, name="z_new2", tag="z_new2", bufs=2)
                layernorm_tile(ln_in, z_new)
                nc.sync.dma_start(out=z_ln2[it][tp * 128:(tp + 1) * 128, :], in_=z_new)
                yield 40
                for c in transpose_z(z_new, tp, ztA_next, None, tag="psf"):
                    yield c

        for it in range(ITERS):
            last = it == ITERS - 1
            if not last:
                ztA_next = ztp.tile([128, ND, T], F8, name="ztAn", tag="ztA")
                zt1h = zt1p.tile([128, ND, T], F8, name="zt1h", tag="zt1h")
                zt1l = zt1p.tile([128, ND, T], F8, name="zt1l", tag="zt1l")
            else:
                ztA_next = zt1h = zt1l = None

            # ---- projections (DR over dp pairs) ----
            def qk_hep2(wr, dst_pack, hep, zt_src):
                stg = qks_p.tile([128, NTH, 512], F8, name="stg", tag="stg")
                for tqh in range(NTH):
                    acc = ps.tile([128, 512], F32, name="acq", tag="ps", bufs=2)
                    for dpp in range(ND // 2):
                        nc.tensor.matmul(acc, lhsT=wr[:, 2 * dpp:2 * dpp + 2,
                                                      hep * 128:(hep + 1) * 128],
                                         rhs=zt_src[:, 2 * dpp:2 * dpp + 2,
                                                    tqh * 512:(tqh + 1) * 512],
                                         start=(dpp == 0), stop=(dpp == ND // 2 - 1),
                                         perf_mode=DRM)
                        yield 110
                    nc.vector.tensor_copy(out=stg[:, tqh, :], in_=acc)
                    yield 0
                # weight cols are block-swapped so stg partitions [64*sub, +64)
                # hold (h0 dims 32sub.., h1 dims 32sub..) -> 2 contiguous DMAs
                base = 64 * (hep % 2)
                for sub in range(2):
                    nc.sync.dma_start(
                        out=dst_pack[base:base + 64, sub, hep // 2, :],
                        in_=stg[64 * sub:64 * sub + 64, :, :])
                yield 40

            def v_blk(sp, half, zt_src):
                acc = ps.tile([128, 512], F32, name="acv", tag="ps", bufs=2)
                for dpp in range(ND // 2):
                    nc.tensor.matmul(
                        acc,
                        lhsT=zt_src[:, 2 * dpp:2 * dpp + 2, sp * 128:(sp + 1) * 128],
                        rhs=wv_r[:, 2 * dpp:2 * dpp + 2, half * 512:(half + 1) * 512],
                        start=(dpp == 0), stop=(dpp == ND // 2 - 1), perf_mode=DRM)
                    yield 110
                nc.vector.tensor_copy(
                    out=vg[:, sp, half * 8:(half + 1) * 8, :],
                    in_=acc.rearrange("p (h e) -> p h e", e=64))
                yield 40

            def run_gens(gens):
                for g in gens:
                    for c in g:
                        pump(c)

            qk_gens = deque()
            for hep in range(NHEP):
                qk_gens.append([qk_hep2(wq_r, qt_pack, hep, ztA),
                                qk_hep2(wk_r, kt_pack, hep, ztA)])
            v_gens = deque()
            for half in range(2):
                for sp in range(NT1):
                    v_gens.append(v_blk(sp, half, ztA))
            run_gens(qk_gens.popleft())
            run_gens(qk_gens.popleft())
            run_gens(qk_gens.popleft())

            # ---- attention core ----
            z_prev = z_in if it == 0 else z_ln2[it - 1]
            dst_ln1 = z_out if last else z_ln1[it]
            state = {}

            def attn_scores_exp(tq, hep):
                et = et_p.tile([128, NT1, 2, 256], F8, name="et", tag="et")
                state["et"] = et
                for g in range(NT1 // 4):
                    for hh in range(2):
                        # one tile_position per PSUM tile (HW constraint)
                        sc = ps.tile([128, 4, 256], F32, name="sc", tag="sc", bufs=2)
                        Bp = 32 * (2 * (hep % 2) + hh)
                        hslot = hep // 2
                        for spi in range(4):
                            sp = 4 * g + spi
                            nc.tensor.matmul(
                                sc[:, spi, :],
                                lhsT=kt_pack[Bp:Bp + 32, :, hslot,
                                             sp * 128:(sp + 1) * 128],
                                rhs=qt_pack[Bp:Bp + 32, :, hslot,
                                            tq * 256:(tq + 1) * 256],
                                start=True, stop=True, perf_mode=DRM,
                                tile_position=(Bp, 0))
                            yield 55
                        nc.scalar.activation(
                            out=et[:, 4 * g:4 * g + 4, hh, :],
                            in_=sc, func=AF.Exp, scale=EXP_SCALE)
                        yield 1400

            def attn_pv(tq, hep, et, outcat):
                pvd = ps.tile([128, 2, 2, 2, 64], F32, name="pvd", tag="ps", bufs=2)
                pvt = pvd[:, 0]
                den = pvd[:, 1]
                for tqc in range(2):
                    for hh in range(2):
                        for spp in range(NT1 // 2):
                            nc.tensor.matmul(
                                pvt[:, tqc, hh, :],
                                lhsT=et[:, 2 * spp:2 * spp + 2, hh,
                                        tqc * 128:(tqc + 1) * 128],
                                rhs=vg[:, 2 * spp:2 * spp + 2, 2 * hep + hh, :],
                                start=(spp == 0), stop=(spp == NT1 // 2 - 1),
                                perf_mode=DRM)
                            yield 28
                        for spp in range(NT1 // 2):
                            nc.tensor.matmul(
                                den[:, tqc, hh, 0:1],
                                lhsT=et[:, 2 * spp:2 * spp + 2, hh,
                                        tqc * 128:(tqc + 1) * 128],
                                rhs=ones2,
                                start=(spp == 0), stop=(spp == NT1 // 2 - 1),
                                perf_mode=DRM)
                            yield 4
                rec = stats.tile([128, 2, 2], F32, name="rec", tag="rec")
                nc.vector.reciprocal(out=rec, in_=den[:, :, :, 0])
                ocn = ocn_p.tile([128, 2, 2, 64], F16, name="ocn", tag="ocn")
                nc.vector.tensor_tensor(
                    out=ocn, in0=pvt[:, :, :, :],
                    in1=rec.rearrange("p a b -> p a b ()").broadcast_to(
                        (128, 2, 2, 64)),
                    op=OP.mult)
                # one 128-wide transpose per tqc: out partitions = (hh, e)
                tr = ps.tile([128, 2, 128], F16, name="tr", tag="ps", bufs=2,
                             padded_shape=[128, 2, 256])
                for tqc in range(2):
                    nc.tensor.transpose(tr[:, tqc, :], in_=ocn[:, tqc, :, :],
                                        identity=ident)
                    yield 55
                nc.vector.tensor_copy(
                    out=outcat[:, hep, :].rearrange("p (c t) -> p c t", c=2),
                    in_=tr)
                yield 0

            def wo_ln(tq, outcat, pend):
                if it == 0:
                    zp2t = work.tile([128, 2, D], F32R, name="zp", tag="zres", bufs=1)
                else:
                    zp2t = work.tile([128, 2, D], F16, name="zp16", tag="zres16", bufs=2)
                nc.sync.dma_start(
                    out=zp2t,
                    in_=z_prev[tq * 256:(tq + 1) * 256, :].rearrange(
                        "(two p) d -> p two d", two=2))
                zps = [zp2t[:, 0, :], zp2t[:, 1, :]]
                for tpq in range(2):
                    accs = []
                    for dq in range(2):
                        accs.append(ps.tile([128, 512], F32, name="awo", tag="ps",
                                            bufs=2))
                    for dq in range(2):
                        for hpp in range(NHEP // 2):
                            nc.tensor.matmul(
                                accs[dq],
                                lhsT=outcat[:, 2 * hpp:2 * hpp + 2,
                                            tpq * 128:(tpq + 1) * 128],
                                rhs=wo_r[:, 2 * hpp:2 * hpp + 2,
                                         dq * 512:(dq + 1) * 512],
                                start=(hpp == 0), stop=(hpp == NHEP // 2 - 1),
                                perf_mode=DRM)
                            yield 110
                    tp = tq * 2 + tpq
                    zp = zps[tpq]
                    ln_in = lnc_p.tile([128, D], F32, name="ln_in", tag="lnc")
                    for dq in range(2):
                        nc.vector.tensor_add(out=ln_in[:, dq * 512:(dq + 1) * 512],
                                             in0=zp[:, dq * 512:(dq + 1) * 512],
                                             in1=accs[dq])
                    z_new = work.tile([128, D], F16, name="z_new", tag="z_new", bufs=3)
                    layernorm_tile(ln_in, z_new)
                    nc.sync.dma_start(out=dst_ln1[tp * 128:(tp + 1) * 128, :], in_=z_new)
                    yield 40
                    if not last:
                        pend.append((z_new, tp))

            def zt1_tr_gen(pend):
                for z_new, tp in pend:
                    for c in transpose_z(z_new, tp, zt1h, zt1l):
                        yield c

            if K_STAGE == "proj":
                dummy = work.tile([128, D], F16, name="dummy", tag="z_new", bufs=3)
                nc.vector.memset(dummy, 0.0)
                for tp in range(NT1):
                    nc.sync.dma_start(out=z_out[tp * 128:(tp + 1) * 128, :], in_=dummy)
                break
            for tq in range(NTQ):
                outcat = outcat_p.tile([128, NHEP, 256], F8, name="outcat", tag="outcat")
                prev_pv = None
                for hep in range(NHEP):
                    if qk_gens:
                        run_gens(qk_gens.popleft())   # hep+2 lookahead
                    for c in attn_scores_exp(tq, hep):
                        pump(c)
                    if v_gens:
                        for _ in range(8):
                            if v_gens:
                                for c in v_gens.popleft():
                                    pump(c)
                    et = state["et"]
                    if K_STAGE in ("scores", "scmm"):
                        continue
                    if prev_pv is not None:
                        for c in prev_pv:
                            pump(c)
                    prev_pv = attn_pv(tq, hep, et, outcat)
                    pump(2000)
                if prev_pv is not None:
                    for c in prev_pv:
                        pump(c)
                if K_STAGE in ("scores", "scmm", "pv"):
                    continue
                pend = []
                for c in wo_ln(tq, outcat, pend):
                    pump(c)
                if not last:
                    fillers.appendleft(zt1_tr_gen(pend))
                    fillers.append(ffn_th(it, tq, zt1h, zt1l, ztA_next,
                                          late=(tq >= NTQ - 2)))
            if K_STAGE in ("scores", "scmm", "pv"):
                dummy = work.tile([128, D], F16, name="dummy", tag="z_new", bufs=3)
                nc.vector.memset(dummy, 0.0)
                for tp in range(NT1):
                    nc.sync.dma_start(out=z_out[tp * 128:(tp + 1) * 128, :], in_=dummy)
                break

            if last:
                break
            drain_all()
            ztA = ztA_next

    nc.compile()
    return nc


def _prep_weights(Wq, Wk, Wv, Wo, W1, W2):
    def pm_e4(a):
        R, C = a.shape
        return np.ascontiguousarray(
            np.ascontiguousarray(a).reshape(R // 128, 128, C).transpose(1, 0, 2))

    def flat(w):
        return np.ascontiguousarray(
            w.transpose(1, 0, 2).reshape(D, D)).astype(np.float32)

    # swap 32-col blocks 1<->2 within each 128-col hep group so the q/k
    # stg partitions become [sub0: h0 d0-31 | h1 d0-31 ; sub1: h0 d32-63 | h1 d32-63]
    # and each pack is 2 contiguous-partition DMAs
    perm = np.concatenate([hep * 128 + np.concatenate(
        [np.arange(0, 32), np.arange(64, 96), np.arange(32, 64), np.arange(96, 128)])
        for hep in range(H // 2)])
    wq8 = pm_e4((flat(Wq)[:, perm] * WS).astype(E4))
    wk8 = pm_e4((flat(Wk)[:, perm] * WS).astype(E4))
    wv8 = pm_e4((flat(Wv) * WS).astype(E4))
    wo8 = pm_e4(Wo.astype(np.float32).astype(E4))      # unscaled
    W1s = W1.astype(np.float32) * WS
    W1hi = W1s.astype(E4)
    W1lo = (W1s - W1hi.astype(np.float32)).astype(E4)
    W2s = W2.astype(np.float32) * WS
    W2hi = W2s.astype(E4)
    W2lo = (W2s - W2hi.astype(np.float32)).astype(E4)
    return (wq8, wk8, wv8, wo8, pm_e4(W1hi), pm_e4(W1lo), pm_e4(W2hi), pm_e4(W2lo))


def kernel(**inputs):
    z = np.asarray(inputs["z"], dtype=np.float32)
    for nm in ("bq", "bk", "bv", "bo", "b1", "b2", "be1", "be2"):
        assert not np.any(np.asarray(inputs[nm])), f"{nm} must be zero (specialized kernel)"
    for nm in ("g1", "g2"):
        assert np.all(np.asarray(inputs[nm]) == 1.0), f"{nm} must be ones (specialized kernel)"

    T = z.shape[1]
    raw_ws = tuple(np.asarray(inputs[nm]) for nm in ("Wq", "Wk", "Wv", "Wo", "W1", "W2"))
    ent = _CACHE.get(T)
    if ent is None or not all(np.array_equal(a, b) for a, b in zip(ent[1], raw_ws)):
        prepped = _prep_weights(*raw_ws)
        ent = (build(T, *prepped), tuple(np.copy(w) for w in raw_ws))
        _CACHE[T] = ent
    nc = ent[0]

    in_maps = [{"z_in": np.ascontiguousarray(z[c])} for c in range(B)]
    res = run_bass_kernel_spmd(nc, in_maps, core_ids=list(range(B)))
    return np.stack([res.results[c]["z_out"] for c in range(B)]).astype(np.float32)
